# revision 1
# baseline (speedup 1.0000x reference)
"""Trainium2 Bass kernel for a dense transformer block (LN->QKV->causal attn->proj
-> residual -> LN -> MLP(gelu tanh) -> residual).

Sharding: 8 independent cores = 4 batches x 2 query-halves. No collectives.
Each core processes its 512 query rows against a locally reordered KV sequence
(diagonal 512 rows first, prefix context after; zero padding + data mask for the
lower half), so every core runs an identical instruction stream and the causal
mask is a compile-time affine_select. Key validity masking is folded into
zeroing V rows (and the softmax-denominator ones column) via per-core data.

All matmuls run as fp32r (full PE rate). Attention probs / QT / KT / V and the
fc2 operands are bf16 (fp32 accumulation in PSUM).
"""

import math
import sys
from dataclasses import dataclass

import numpy as np

sys.path.insert(0, "/opt/trn_rl_repo")

import concourse.bacc as bacc  # noqa: E402
import concourse.bass as bass  # noqa: E402
import concourse.tile as tile  # noqa: E402
from concourse import mybir  # noqa: E402

F32 = mybir.dt.float32
BF16 = mybir.dt.bfloat16
F32R = mybir.dt.float32r
AF = mybir.ActivationFunctionType
ALU = mybir.AluOpType

EPS = 1e-5
DH = 64  # head dim (fixed)


@dataclass(frozen=True)
class Cfg:
    Sq: int = 512     # query rows per core
    Skv: int = 1024   # local KV rows per core
    D: int = 1280     # model dim
    H: int = 20       # heads
    Dff: int = 5120   # MLP hidden

    @property
    def ND(self):
        return self.D // 128      # D chunks

    @property
    def NKB(self):
        return self.Skv // 128    # kv blocks

    @property
    def NDIAG(self):
        return self.Sq // 128     # diagonal kv blocks (local causal)

    @property
    def NQB(self):
        return self.Sq // 128     # query row blocks

    @property
    def NCC(self):
        return (self.H * 65 + 127) // 128  # proj contraction chunks (padded)

    @property
    def NHT(self):
        return self.Dff // 128    # MLP hidden blocks

    @property
    def NG(self):
        return self.D // math.gcd(512, self.D)  # bn_stats subgroups


def tiles_of(total, maxw=512):
    out = []
    c = 0
    while c < total:
        w = min(maxw, total - c)
        out.append((c, w))
        c += w
    return out


def seg_rows(r0, n):
    """Split rows [r0, r0+n) of a [128, NCC, ...] strip at 128 boundaries.
    Yields (chunk, part0, off, cnt)."""
    off = 0
    while off < n:
        r = r0 + off
        cc, p = divmod(r, 128)
        cnt = min(n - off, 128 - p)
        yield cc, p, off, cnt
        off += cnt


def build_program(cfg: Cfg, has_bqk: bool, has_bv: bool, has_bfc2: bool, repeat: int = 1):
    c = cfg
    nc = bacc.Bacc("TRN2", target_bir_lowering=False, debug=False, num_devices=8)

    hid_d = nc.dram_tensor("hid", [c.Skv, c.D], F32, kind="ExternalInput").ap()
    kvm_d = nc.dram_tensor("kvmask", [c.NKB, 128], F32, kind="ExternalInput").ap()
    wqkv_d = nc.dram_tensor("wqkv", [c.D, 3 * c.D], F32R, kind="ExternalInput").ap()
    wproj_d = nc.dram_tensor("wproj", [c.NCC * 128, c.D], F32R, kind="ExternalInput").ap()
    wfc_d = nc.dram_tensor("wfc", [c.D, c.Dff], F32R, kind="ExternalInput").ap()
    wfc2_d = nc.dram_tensor("wfc2", [c.Dff, c.D], BF16, kind="ExternalInput").ap()
    bfc_d = nc.dram_tensor("bfc", [c.NHT, 128], F32, kind="ExternalInput").ap()
    ident_d = nc.dram_tensor("ident", [128, 128], F32R, kind="ExternalInput").ap()
    out_d = nc.dram_tensor("out", [c.Sq, c.D], F32, kind="ExternalOutput").ap()
    bqk_d = bv_d = bfc2_d = None
    if has_bqk:
        bqk_d = nc.dram_tensor("bqk", [2 * c.ND, 128], F32, kind="ExternalInput").ap()
    if has_bv:
        bv_d = nc.dram_tensor("bv", [c.H, 64], F32, kind="ExternalInput").ap()
    if has_bfc2:
        bfc2_d = nc.dram_tensor("bfc2", [1, c.D], F32, kind="ExternalInput").ap()

    with tile.TileContext(nc) as tc, nc.allow_low_precision(
        reason="fp32r tiles hold fp32 bits; PE rounds internally"
    ):
        # -------- pools alive the whole kernel --------
        constp = tc.alloc_tile_pool(name="constp", bufs=1)
        workp = tc.alloc_tile_pool(name="workp", bufs=2)
        statp = tc.alloc_tile_pool(name="statp", bufs=2)
        wcache = tc.alloc_tile_pool(name="wcache", bufs=22)
        pb = tc.alloc_tile_pool(name="pb", bufs=6, space="PSUM")
        pt = tc.alloc_tile_pool(name="pt", bufs=2, space="PSUM")

        # constants
        ident_sb = constp.tile([128, 128], F32R, name="ident_sb")
        nc.sync.dma_start(out=ident_sb, in_=ident_d)
        kvm_sb = constp.tile([128, c.NKB], F32, name="kvm_sb")
        nc.sync.dma_start(out=kvm_sb, in_=kvm_d.rearrange("i p -> p i"))
        eps_sb = constp.tile([128, 1], F32, name="eps_sb")
        nc.vector.memset(eps_sb, EPS)
        warm_sb = constp.tile([128, 1], F32, name="warm_sb")
        nc.scalar.activation(out=warm_sb, in_=eps_sb, func=AF.Sqrt)
        ones_h = constp.tile([128, c.H, 1], BF16, name="ones_h")
        nc.vector.memset(ones_h, 1.0)
        bfc_sb = constp.tile([128, c.NHT], F32, name="bfc_sb")
        nc.sync.dma_start(out=bfc_sb, in_=bfc_d.rearrange("i p -> p i"))
        bqk_sb = bv_sb = bfc2_sb = None
        if has_bqk:
            bqk_sb = constp.tile([128, 2 * c.ND], F32, name="bqk_sb")
            nc.sync.dma_start(out=bqk_sb, in_=bqk_d.rearrange("i p -> p i"))
        if has_bv:
            bv_sb = constp.tile([64, c.H], F32, name="bv_sb")
            nc.sync.dma_start(out=bv_sb, in_=bv_d.rearrange("h c -> c h"))
        if has_bfc2:
            bfc2_sb = constp.tile([128, c.D], F32, name="bfc2_sb")
            nc.gpsimd.dma_start(
                out=bfc2_sb,
                in_=bass.AP(
                    tensor=bfc2_d.tensor,
                    offset=bfc2_d.offset,
                    ap=[[0, 128], bfc2_d.ap[1]],
                ),
            )

        def emit_body(rep):
            _emit_phases(rep)

        def layer_norm_tile(x_ap, xhat_ap, i):
            """row LN: xhat = (x - mean) * rsqrt(var + eps)."""
            stats = statp.tile([128, c.NG, 6], F32, name=f"stats_{i}", tag="stats")
            xg = x_ap.rearrange("p (g d) -> p g d", g=c.NG)
            for g in range(c.NG):
                nc.vector.bn_stats(out=stats[:, g, :], in_=xg[:, g, :])
            mv = statp.tile([128, 2], F32, name=f"mv_{i}", tag="mv")
            nc.vector.bn_aggr(out=mv, in_=stats)
            nc.scalar.activation(
                out=mv[:, 1:2], in_=mv[:, 1:2], func=AF.Sqrt, bias=eps_sb
            )
            nc.vector.reciprocal(out=mv[:, 1:2], in_=mv[:, 1:2])
            nc.vector.tensor_scalar(
                out=xhat_ap,
                in0=x_ap,
                scalar1=mv[:, 0:1],
                scalar2=mv[:, 1:2],
                op0=ALU.subtract,
                op1=ALU.mult,
            )

        def transpose_128(src_ap, dst_ap):
            ps = pt.tile([128, 128], F32R, name="ps_t", tag="ps_t")
            nc.tensor.transpose((ps), (src_ap), (ident_sb))
            nc.vector.tensor_copy(out=dst_ap, in_=ps)

        def _emit_phases(rep):
            residp = tc.alloc_tile_pool(name=f"residp{rep}", bufs=1, side="right")
            resid = residp.tile([128, c.NQB, c.D], F32, name=f"resid{rep}")
            # -------- pools: phase A..B --------
            qkvp = tc.alloc_tile_pool(name=f"qkvp{rep}", bufs=1)
            QT = qkvp.tile([128, c.ND, c.Sq], BF16, name="QT")
            KT = qkvp.tile([128, c.ND, c.Skv], BF16, name="KT")
            V = qkvp.tile([128, c.NKB, c.H, 65], BF16, name="V")

            xlp = tc.alloc_tile_pool(name=f"xlp{rep}", bufs=1)  # phase A only
            xlnT = xlp.tile([128, c.ND, c.Skv], F32R, name="xlnT")

            # ---- Phase A: LN1 + transpose + QKV ----
            for i in range(c.NKB):
                if i < c.NQB:
                    x_t = resid[:, i, :]
                else:
                    x_t = workp.tile([128, c.D], F32, name="x_t", tag="row")
                nc.sync.dma_start(out=x_t, in_=hid_d[i * 128:(i + 1) * 128, :])
                xhat = workp.tile([128, c.D], F32R, name="xhat", tag="row2")
                layer_norm_tile(x_t, xhat, i)
                for d in range(c.ND):
                    transpose_128(
                        xhat[:, d * 128:(d + 1) * 128],
                        xlnT[:, d, i * 128:(i + 1) * 128],
                    )

            # V (natural layout) + kv masking + ones column
            for c0, w in tiles_of(c.D):
                wts = []
                for d in range(c.ND):
                    wv = wcache.tile([128, 512], F32R, name="wv", tag="wc")
                    nc.sync.dma_start(
                        out=wv[:, :w],
                        in_=wqkv_d[d * 128:(d + 1) * 128,
                                   2 * c.D + c0: 2 * c.D + c0 + w],
                    )
                    wts.append(wv)
                h0 = c0 // 64
                nh = w // 64
                for i in range(c.NKB):
                    ps = pb.tile([128, 512], F32, name="ps_v", tag="pb")
                    for d in range(c.ND):
                        nc.tensor.matmul(
                            ps[:, :w],
                            lhsT=(xlnT[:, d, i * 128:(i + 1) * 128]),
                            rhs=(wts[d][:, :w]),
                            start=(d == 0),
                            stop=(d == c.ND - 1),
                        )
                    nc.scalar.activation(
                        out=V[:, i, h0:h0 + nh, 0:64],
                        in_=ps[:, :w].rearrange("p (h x) -> p h x", x=64),
                        func=AF.Copy,
                        scale=kvm_sb[:, i:i + 1],
                    )
            for i in range(c.NKB):
                nc.vector.tensor_scalar_mul(
                    out=V[:, i, :, 64:65],
                    in0=ones_h,
                    scalar1=kvm_sb[:, i:i + 1],
                )

            # Q and K (transposed outputs), o-blocks in pairs
            for g0, gw in tiles_of(c.D):
                for which, base, dst, ncol in (
                    ("q", 0, QT, c.Sq),
                    ("k", c.D, KT, c.Skv),
                ):
                    wts = []
                    for d in range(c.ND):
                        w2 = wcache.tile([128, 512], F32R, name=f"w2_{which}", tag="wc")
                        nc.sync.dma_start(
                            out=w2[:, :gw],
                            in_=wqkv_d[
                                d * 128:(d + 1) * 128,
                                base + g0: base + g0 + gw,
                            ],
                        )
                        wts.append(w2)
                    for j in range(gw // 128):
                        o = g0 // 128 + j
                        for c0, w in tiles_of(ncol):
                            ps = pb.tile([128, 512], F32, name="ps_qk", tag="pb")
                            for d in range(c.ND):
                                nc.tensor.matmul(
                                    ps[:, :w],
                                    lhsT=(wts[d][:, j * 128:(j + 1) * 128]),
                                    rhs=(xlnT[:, d, c0:c0 + w]),
                                    start=(d == 0),
                                    stop=(d == c.ND - 1),
                                )
                            nc.vector.tensor_copy(
                                out=dst[:, o, c0:c0 + w],
                                in_=ps[:, :w],
                            )
                            if has_bqk:
                                col = o if which == "q" else c.ND + o
                                nc.vector.tensor_scalar_add(
                                    out=dst[:, o, c0:c0 + w],
                                    in0=dst[:, o, c0:c0 + w],
                                    scalar1=bqk_sb[:, col:col + 1],
                                )

            xlp.release()  # xlnT dead

            # -------- pools: phase B (attention) --------
            ytp = tc.alloc_tile_pool(name=f"ytp{rep}", bufs=1, side="right")
            expp = tc.alloc_tile_pool(name=f"expp{rep}", bufs=3, side="right")
            ynp = tc.alloc_tile_pool(name=f"ynp{rep}", bufs=2, side="right")
            drp = tc.alloc_tile_pool(name=f"drp{rep}", bufs=1, space="DRAM")
            dscr = drp.tile([c.H, c.Sq], F32R, name=f"dscr{rep}")
            yT = ytp.tile([128, c.NCC, c.Sq], F32R, name="yT")
            for cc, p0, off, cnt in seg_rows(65 * c.H, c.NCC * 128 - 65 * c.H):
                nc.sync.dma_start(
                    out=yT[p0:p0 + cnt, cc, :],
                    in_=wproj_d[65 * c.H + off:65 * c.H + off + cnt, 0:c.Sq],
                )

            fill0 = nc.gpsimd.to_reg(0.0)
            for h in range(c.H):
                pbase = 64 * (h % 2)
                o = h // 2
                eT = expp.tile([128, c.NKB, c.Sq], BF16, name="eT", tag="eT")
                yps = pb.tile([128, 512], F32, name="yps", tag="pb")
                for kb in range(c.NKB):
                    sps = pb.tile([128, 512], F32, name="sps", tag="pb")
                    nc.tensor.matmul(
                        sps[:, :c.Sq],
                        lhsT=KT[pbase:pbase + 64, o,
                                kb * 128:(kb + 1) * 128],
                        rhs=QT[pbase:pbase + 64, o, :],
                        start=True,
                        stop=True,
                    )
                    nc.scalar.activation(
                        out=eT[:, kb, :],
                        in_=sps[:, :c.Sq],
                        func=AF.Exp,
                        scale=1.0 / math.sqrt(DH),
                    )
                    if kb < c.NDIAG:
                        ncol = 128 * (kb + 1)
                        nc.gpsimd.affine_select(
                            out=eT[:, kb, 0:ncol],
                            in_=eT[:, kb, 0:ncol],
                            pattern=[[1, ncol]],
                            compare_op=ALU.is_ge,
                            fill=fill0,
                            base=-128 * kb,
                            channel_multiplier=-1,
                        )
                av_order = list(range(c.NDIAG, c.NKB)) + list(range(c.NDIAG))
                for n_i, kb in enumerate(av_order):
                    nc.tensor.matmul(
                        yps[0:65, :c.Sq],
                        lhsT=V[:, kb, h, :],
                        rhs=eT[:, kb, :],
                        start=(n_i == 0),
                        stop=(n_i == c.NKB - 1),
                    )
                # normalize: rows 0..63 / row 64
                yTu = ynp.tile([65, c.Sq], F32R, name="yTu", tag="yTu")
                nc.vector.tensor_copy(out=yTu, in_=yps[0:65, :c.Sq])
                nc.vector.reciprocal(out=yTu[64:65, :], in_=yps[64:65, :c.Sq])
                rbs = ynp.tile([65, c.Sq], F32R, name="rbs", tag="rbs")
                nc.sync.dma_start(out=dscr[h:h + 1, :], in_=yTu[64:65, :])
                _src = dscr[h:h + 1, :]
                nc.gpsimd.dma_start(
                    out=rbs,
                    in_=bass.AP(
                        tensor=_src.tensor,
                        offset=_src.offset,
                        ap=[[0, 65]] + list(_src.ap[1:]),
                    ),
                )
                yTn = ynp.tile([65, c.Sq], F32R, name="yTn", tag="yTn")
                nc.vector.tensor_mul(
                    out=yTn[0:65, :], in0=yTu[0:65, :], in1=rbs[0:65, :]
                )
                if has_bv:
                    nc.vector.tensor_scalar_add(
                        out=yTn[0:64, :],
                        in0=yTn[0:64, :],
                        scalar1=bv_sb[:, h:h + 1],
                    )
                for cc, p0, off, cnt in seg_rows(65 * h, 65):
                    nc.sync.dma_start(
                        out=yT[p0:p0 + cnt, cc, :],
                        in_=yTn[off:off + cnt, :],
                    )

            drp.release()
            ynp.release()
            expp.release()
            qkvp.release()  # QT/KT/V dead

            # -------- pools: phase C..D --------
            hzp = tc.alloc_tile_pool(name=f"hzp{rep}", bufs=1)
            h_sb = hzp.tile([128, c.NQB, c.D], F32, name="h_sb")
            zT = hzp.tile([128, c.ND, c.Sq], F32R, name="zT")

            # ---- Phase C: proj + residual + LN2 + transpose ----
            for c0, w in tiles_of(c.D):
                wts = []
                for cc in range(c.NCC):
                    wp = wcache.tile([128, 512], F32R, name="wp", tag="wc")
                    nc.sync.dma_start(
                        out=wp[:, :w],
                        in_=wproj_d[cc * 128:(cc + 1) * 128, c0:c0 + w],
                    )
                    wts.append(wp)
                for qb in range(c.NQB):
                    ps = pb.tile([128, 512], F32, name="ps_p", tag="pb")
                    for cc in range(c.NCC):
                        nc.tensor.matmul(
                            ps[:, :w],
                            lhsT=(yT[:, cc, qb * 128:(qb + 1) * 128]),
                            rhs=(wts[cc][:, :w]),
                            start=(cc == 0),
                            stop=(cc == c.NCC - 1),
                        )
                    nc.vector.tensor_add(
                        out=h_sb[:, qb, c0:c0 + w],
                        in0=ps[:, :w],
                        in1=resid[:, qb, c0:c0 + w],
                    )

            ytp.release()  # yT dead
            residp.release()

            for qb in range(c.NQB):
                xhat2 = workp.tile([128, c.D], F32R, name="xhat2", tag="row2")
                layer_norm_tile(h_sb[:, qb, :], xhat2, 100 + qb)
                for d in range(c.ND):
                    transpose_128(
                        xhat2[:, d * 128:(d + 1) * 128],
                        zT[:, d, qb * 128:(qb + 1) * 128],
                    )

            # ---- Phase D: MLP ----
            gp = tc.alloc_tile_pool(name=f"gp{rep}", bufs=1)
            outp = tc.alloc_tile_pool(name=f"outp{rep}", bufs=1)
            w2p = tc.alloc_tile_pool(name=f"w2p{rep}", bufs=6)
            gT = gp.tile([128, c.NHT, c.Sq], BF16, name="gT")

            for g0, gw in tiles_of(c.Dff):
                wts = []
                for d in range(c.ND):
                    wf = wcache.tile([128, 512], F32R, name="wf", tag="wc")
                    nc.sync.dma_start(
                        out=wf[:, :gw],
                        in_=wfc_d[d * 128:(d + 1) * 128, g0:g0 + gw],
                    )
                    wts.append(wf)
                for j in range(gw // 128):
                    ht = g0 // 128 + j
                    ps = pb.tile([128, 512], F32, name="ps_f", tag="pb")
                    for d in range(c.ND):
                        nc.tensor.matmul(
                            ps[:, :c.Sq],
                            lhsT=(wts[d][:, j * 128:(j + 1) * 128]),
                            rhs=(zT[:, d, :]),
                            start=(d == 0),
                            stop=(d == c.ND - 1),
                        )
                    nc.scalar.activation(
                        out=gT[:, ht, :],
                        in_=ps[:, :c.Sq],
                        func=AF.Gelu_apprx_tanh,
                        bias=bfc_sb[:, ht:ht + 1],
                    )

            out_ts = []
            for qs in range(c.NQB):
                ot = outp.tile([128, c.D], F32, name=f"out_{qs}")
                out_ts.append(ot)
            for c0, w in tiles_of(c.D):
                psms = [
                    pb.tile([128, 512], F32, name=f"ps_m_{qs}", tag="pb")
                    for qs in range(c.NQB)
                ]
                for ht in range(c.NHT):
                    wf2 = w2p.tile([128, 512], BF16, name="wf2", tag="wf2")
                    nc.sync.dma_start(
                        out=wf2[:, :w],
                        in_=wfc2_d[ht * 128:(ht + 1) * 128, c0:c0 + w],
                    )
                    for qs in range(c.NQB):
                        nc.tensor.matmul(
                            psms[qs][:, :w],
                            lhsT=gT[:, ht, qs * 128:(qs + 1) * 128],
                            rhs=wf2[:, :w],
                            start=(ht == 0),
                            stop=(ht == c.NHT - 1),
                        )
                for qs in range(c.NQB):
                    nc.vector.tensor_add(
                        out=out_ts[qs][:, c0:c0 + w],
                        in0=psms[qs][:, :w],
                        in1=h_sb[:, qs, c0:c0 + w],
                    )
            for qs in range(c.NQB):
                if has_bfc2:
                    nc.vector.tensor_add(
                        out=out_ts[qs], in0=out_ts[qs], in1=bfc2_sb
                    )
                nc.sync.dma_start(
                    out=out_d[qs * 128:(qs + 1) * 128, :],
                    in_=out_ts[qs],
                )


            w2p.release()
            outp.release()
            gp.release()
            hzp.release()

        for _rep in range(repeat):
            emit_body(_rep)

        pt.release()
        pb.release()
        wcache.release()
        statp.release()
        workp.release()
        constp.release()

    nc.compile()
    return nc


# ----------------------------------------------------------------------------
# Host-side preparation
# ----------------------------------------------------------------------------

def prep_inputs(cfg: Cfg, hidden_states, attention_mask, ln1_g, ln1_b, w_qkv,
                b_qkv, w_proj, b_proj, ln2_g, ln2_b, w_fc, b_fc, w_fc2, b_fc2):
    """Build per-core in_maps. Returns (in_maps, flags)."""
    c = cfg
    B = hidden_states.shape[0]
    f32 = np.float32

    # fold LN affine params into following matmuls
    wqkv_f = (ln1_g[:, None] * w_qkv).astype(f32)
    bqkv_f = (ln1_b @ w_qkv + b_qkv).astype(f32)
    wfc_f = (ln2_g[:, None] * w_fc).astype(f32)
    bfc_f = (ln2_b @ w_fc + b_fc).astype(f32)

    # proj weight strip with per-head ones rows carrying b_proj/H
    wproj_p = np.zeros((c.NCC * 128, c.D), f32)
    for h in range(c.H):
        wproj_p[65 * h:65 * h + 64, :] = w_proj[64 * h:64 * h + 64, :]
        wproj_p[65 * h + 64, :] = b_proj / c.H

    bq = bqkv_f[0:c.D]
    bk = bqkv_f[c.D:2 * c.D]
    bv = bqkv_f[2 * c.D:3 * c.D]

    has_bqk = bool(np.any(bq) or np.any(bk))
    has_bv = bool(np.any(bv))
    has_bfc2 = bool(np.any(b_fc2))

    import ml_dtypes
    shared = {
        "wqkv": wqkv_f,
        "wproj": wproj_p,
        "wfc": wfc_f,
        "wfc2": np.asarray(w_fc2).astype(ml_dtypes.bfloat16),
        "bfc": bfc_f.reshape(c.NHT, 128),
        "ident": np.eye(128, dtype=f32),
    }
    if has_bqk:
        shared["bqk"] = np.concatenate(
            [bq.reshape(c.ND, 128), bk.reshape(c.ND, 128)], axis=0
        )
    if has_bv:
        shared["bv"] = bv.reshape(c.H, 64)
    if has_bfc2:
        shared["bfc2"] = b_fc2.reshape(1, c.D).astype(f32)

    amask = np.asarray(attention_mask).astype(f32)
    in_maps = []
    for core in range(2 * B):
        b, g = divmod(core, 2)
        q0 = g * c.Sq
        hid = np.zeros((c.Skv, c.D), f32)
        kvm = np.zeros((c.Skv,), f32)
        hid[0:c.Sq] = hidden_states[b, q0:q0 + c.Sq]
        kvm[0:c.Sq] = amask[b, q0:q0 + c.Sq]
        if q0 > 0:
            hid[c.Sq:c.Sq + q0] = hidden_states[b, 0:q0]
            kvm[c.Sq:c.Sq + q0] = amask[b, 0:q0]
        m = dict(shared)
        m["hid"] = hid
        m["kvmask"] = kvm.reshape(c.NKB, 128)
        in_maps.append(m)
    return in_maps, (has_bqk, has_bv, has_bfc2)


_CACHE = {}
LAST_RESULTS = None


def kernel(**inputs):
    global LAST_RESULTS
    import concourse.bass_utils as bass_utils

    cfg = Cfg()
    inputs = {k: np.asarray(v) for k, v in inputs.items()}
    B = inputs["hidden_states"].shape[0]
    in_maps, flags = prep_inputs(cfg, **inputs)

    key = (cfg, flags)
    if key not in _CACHE:
        _CACHE[key] = build_program(cfg, *flags)
    nc = _CACHE[key]

    res = bass_utils.run_bass_kernel_spmd(
        nc, in_maps, core_ids=list(range(2 * B))
    )
    LAST_RESULTS = res

    out = np.zeros((B, 2 * cfg.Sq, cfg.D), np.float32)
    for core in range(2 * B):
        b, g = divmod(core, 2)
        out[b, g * cfg.Sq:(g + 1) * cfg.Sq] = res.results[core]["out"]
    return out



# revision 21
# speedup vs baseline: 1.1568x; 1.1568x over previous
"""Trainium2 Bass kernel for a dense transformer block (LN->QKV->causal attn->proj
-> residual -> LN -> MLP(gelu tanh) -> residual).

Sharding: 8 independent cores = 4 batches x 2 query-halves. No collectives.
Each core processes its 512 query rows against a locally reordered KV sequence
(diagonal 512 rows first, prefix context after; zero padding + data mask for the
lower half), so every core runs an identical instruction stream and the causal
mask is a compile-time affine_select. Key validity masking is folded into
zeroing V rows (and the softmax-denominator ones column) via per-core data.

Datatypes: Q/K projections run as fp8e4m3 DoubleRow matmuls (2 contraction
chunks per pass, 2x PE rate); the fp8 scale factors are folded into the
softmax exp scale. V/proj/fc1/fc2 matmuls run bf16 (full PE rate, half the
weight DMA of fp32). Scores for the diagonal (causal) KV blocks only compute
the valid upper-triangular column range.
"""

import math
import sys
from dataclasses import dataclass

import numpy as np

sys.path.insert(0, "/opt/trn_rl_repo")

import concourse.bacc as bacc  # noqa: E402
import concourse.bass as bass  # noqa: E402
import concourse.tile as tile  # noqa: E402
from concourse import mybir  # noqa: E402

F32 = mybir.dt.float32
BF16 = mybir.dt.bfloat16
F32R = mybir.dt.float32r
F8 = mybir.dt.float8e4
AF = mybir.ActivationFunctionType
ALU = mybir.AluOpType

EPS = 1e-5
DH = 64  # head dim (fixed)

S_X8 = 16.0     # fp8 scale on x-hat (LN output, |x|<~6)
S_W8 = 1024.0   # fp8 scale on q/k weights (|w|<~0.1)
F_QK = S_X8 * S_W8
QK_FP8 = True   # fp8 DoubleRow for Q/K projections (else bf16)
DBG = False     # emit per-stage debug DRAM outputs


@dataclass(frozen=True)
class Cfg:
    Sq: int = 512     # query rows per core
    Skv: int = 1024   # local KV rows per core
    D: int = 1280     # model dim
    H: int = 20       # heads
    Dff: int = 5120   # MLP hidden

    @property
    def ND(self):
        return self.D // 128      # D chunks

    @property
    def NKB(self):
        return self.Skv // 128    # kv blocks

    @property
    def NDIAG(self):
        return self.Sq // 128     # diagonal kv blocks (local causal)

    @property
    def NQB(self):
        return self.Sq // 128     # query row blocks

    @property
    def NCC(self):
        return (self.H * 65 + 127) // 128  # proj contraction chunks (padded)

    @property
    def NHT(self):
        return self.Dff // 128    # MLP hidden blocks

    @property
    def NG(self):
        return self.D // math.gcd(512, self.D)  # bn_stats subgroups


def tiles_of(total, maxw=512):
    out = []
    c = 0
    while c < total:
        w = min(maxw, total - c)
        out.append((c, w))
        c += w
    return out


def seg_rows(r0, n):
    """Split rows [r0, r0+n) of a [128, NCC, ...] strip at 128 boundaries.
    Yields (chunk, part0, off, cnt)."""
    off = 0
    while off < n:
        r = r0 + off
        cc, p = divmod(r, 128)
        cnt = min(n - off, 128 - p)
        yield cc, p, off, cnt
        off += cnt


def build_program(cfg: Cfg, has_bqk: bool, has_bv: bool, has_bfc2: bool, repeat: int = 1):
    c = cfg
    nc = bacc.Bacc("TRN2", target_bir_lowering=False, debug=False, num_devices=8)

    hid_d = nc.dram_tensor("hid", [c.Skv, c.D], F32, kind="ExternalInput").ap()
    kvm_d = nc.dram_tensor("kvmask", [c.NKB, 128], F32, kind="ExternalInput").ap()
    if QK_FP8:
        wqk8_d = nc.dram_tensor("wqk8", [c.ND // 2 * 128, 2, 2 * c.D], F8,
                                kind="ExternalInput").ap()
    else:
        wqk8_d = nc.dram_tensor("wqk", [c.D, 2 * c.D], BF16,
                                kind="ExternalInput").ap()
    wv_d = nc.dram_tensor("wv", [c.D, c.D], BF16, kind="ExternalInput").ap()
    wproj_d = nc.dram_tensor("wproj", [c.NCC * 128, c.D], BF16, kind="ExternalInput").ap()
    wfc_d = nc.dram_tensor("wfc", [c.D, c.Dff], BF16, kind="ExternalInput").ap()
    wfc2_d = nc.dram_tensor("wfc2", [c.Dff, c.D], BF16, kind="ExternalInput").ap()
    bfc_d = nc.dram_tensor("bfc", [c.NHT, 128], F32, kind="ExternalInput").ap()
    ident_d = nc.dram_tensor("ident", [128, 128], F32R, kind="ExternalInput").ap()
    out_d = nc.dram_tensor("out", [c.Sq, c.D], F32, kind="ExternalOutput").ap()
    dbg = {}
    if DBG:
        dbg["qt"] = nc.dram_tensor("dbg_qt", [128, 512], F32, kind="ExternalOutput").ap()
        dbg["kt"] = nc.dram_tensor("dbg_kt", [128, 512], F32, kind="ExternalOutput").ap()
        dbg["x8"] = nc.dram_tensor("dbg_x8", [128, 512], F32, kind="ExternalOutput").ap()
        dbg["xl"] = nc.dram_tensor("dbg_xl", [128, 512], F32, kind="ExternalOutput").ap()
        dbg["et"] = nc.dram_tensor("dbg_et", [128, 8, 512], F32, kind="ExternalOutput").ap()
        dbg["ytu"] = nc.dram_tensor("dbg_ytu", [65, 512], F32, kind="ExternalOutput").ap()
        dbg["rbs"] = nc.dram_tensor("dbg_rbs", [65, 512], F32, kind="ExternalOutput").ap()
        dbg["ytn"] = nc.dram_tensor("dbg_ytn", [65, 512], F32, kind="ExternalOutput").ap()
        dbg["h"] = nc.dram_tensor("dbg_h", [128, 1280], F32, kind="ExternalOutput").ap()
        dbg["yt"] = nc.dram_tensor("dbg_yt", [128, 11, 512], F32, kind="ExternalOutput").ap()
        dbg["resid"] = nc.dram_tensor("dbg_resid", [128, 1280], F32, kind="ExternalOutput").ap()
        dbg["v0"] = nc.dram_tensor("dbg_v0", [128, 65], F32, kind="ExternalOutput").ap()
    bqk_d = bv_d = bfc2_d = None
    if has_bqk:
        bqk_d = nc.dram_tensor("bqk", [2 * c.ND, 128], F32, kind="ExternalInput").ap()
    if has_bv:
        bv_d = nc.dram_tensor("bv", [c.H, 64], F32, kind="ExternalInput").ap()
    if has_bfc2:
        bfc2_d = nc.dram_tensor("bfc2", [1, c.D], F32, kind="ExternalInput").ap()

    with tile.TileContext(nc) as tc, nc.allow_low_precision(
        reason="fp32r tiles hold fp32 bits; PE rounds internally"
    ):
        # -------- pools alive the whole kernel --------
        constp = tc.alloc_tile_pool(name="constp", bufs=1)
        workp = tc.alloc_tile_pool(name="workp", bufs=2)
        statp = tc.alloc_tile_pool(name="statp", bufs=2)
        wcache = tc.alloc_tile_pool(name="wcache", bufs=22)
        pb = tc.alloc_tile_pool(name="pb", bufs=6, space="PSUM")
        pt = tc.alloc_tile_pool(name="pt", bufs=2, space="PSUM")

        # constants
        ident_sb = constp.tile([128, 128], F32R, name="ident_sb")
        nc.sync.dma_start(out=ident_sb, in_=ident_d)
        kvm_sb = constp.tile([128, c.NKB], F32, name="kvm_sb")
        nc.sync.dma_start(out=kvm_sb, in_=kvm_d.rearrange("i p -> p i"))
        eps_sb = constp.tile([128, 1], F32, name="eps_sb")
        nc.vector.memset(eps_sb, EPS)
        warm_sb = constp.tile([128, 1], F32, name="warm_sb")
        nc.scalar.activation(out=warm_sb, in_=eps_sb, func=AF.Sqrt)
        ones_h = constp.tile([128, c.H, 1], BF16, name="ones_h")
        nc.vector.memset(ones_h, 1.0)
        bfc_sb = constp.tile([128, c.NHT], F32, name="bfc_sb")
        nc.sync.dma_start(out=bfc_sb, in_=bfc_d.rearrange("i p -> p i"))
        bqk_sb = bv_sb = bfc2_sb = None
        if has_bqk:
            bqk_sb = constp.tile([128, 2 * c.ND], F32, name="bqk_sb")
            nc.sync.dma_start(out=bqk_sb, in_=bqk_d.rearrange("i p -> p i"))
        if has_bv:
            bv_sb = constp.tile([64, c.H], F32, name="bv_sb")
            nc.sync.dma_start(out=bv_sb, in_=bv_d.rearrange("h c -> c h"))
        if has_bfc2:
            bfc2_sb = constp.tile([128, c.D], F32, name="bfc2_sb")
            nc.gpsimd.dma_start(
                out=bfc2_sb,
                in_=bass.AP(
                    tensor=bfc2_d.tensor,
                    offset=bfc2_d.offset,
                    ap=[[0, 128], bfc2_d.ap[1]],
                ),
            )

        def layer_norm_tile(x_ap, xhat_ap, i):
            """row LN: xhat = (x - mean) * rsqrt(var + eps)."""
            stats = statp.tile([128, c.NG, 6], F32, name=f"stats_{i}", tag="stats")
            xg = x_ap.rearrange("p (g d) -> p g d", g=c.NG)
            for g in range(c.NG):
                nc.vector.bn_stats(out=stats[:, g, :], in_=xg[:, g, :])
            mv = statp.tile([128, 2], F32, name=f"mv_{i}", tag="mv")
            nc.vector.bn_aggr(out=mv, in_=stats)
            nc.scalar.activation(
                out=mv[:, 1:2], in_=mv[:, 1:2], func=AF.Sqrt, bias=eps_sb
            )
            nc.vector.reciprocal(out=mv[:, 1:2], in_=mv[:, 1:2])
            nc.vector.tensor_scalar(
                out=xhat_ap,
                in0=x_ap,
                scalar1=mv[:, 0:1],
                scalar2=mv[:, 1:2],
                op0=ALU.subtract,
                op1=ALU.mult,
            )

        def transpose_128(src_ap, dst_ap, dst8_ap=None):
            ps = pt.tile([128, 128], F32R, name="ps_t", tag="ps_t")
            nc.tensor.transpose((ps), (src_ap), (ident_sb))
            nc.vector.tensor_copy(out=dst_ap, in_=ps)
            if dst8_ap is not None:
                nc.scalar.activation(out=dst8_ap, in_=ps, func=AF.Copy, scale=S_X8)

        def _emit_phases(rep):
            residp = tc.alloc_tile_pool(name=f"residp{rep}", bufs=1, side="right")
            resid = residp.tile([128, c.NQB, c.D], F32, name=f"resid{rep}")
            # -------- pools: phase A..B --------
            qkvp = tc.alloc_tile_pool(name=f"qkvp{rep}", bufs=1)
            QT = qkvp.tile([128, c.ND, c.Sq], BF16, name="QT")
            KT = qkvp.tile([128, c.ND, c.Skv], BF16, name="KT")
            V = qkvp.tile([128, c.NKB, c.H, 65], BF16, name="V")

            xlp = tc.alloc_tile_pool(name=f"xlp{rep}", bufs=1)  # phase A only
            xlnT = xlp.tile([128, c.ND, c.Skv], BF16, name="xlnT")
            xlnT8 = xlp.tile([128, c.ND, c.Skv], F8, name="xlnT8")

            # ---- Phase A: LN1 + transpose + QKV ----
            for i in range(c.NKB):
                if i < c.NQB:
                    x_t = resid[:, i, :]
                else:
                    x_t = workp.tile([128, c.D], F32, name="x_t", tag="row")
                nc.sync.dma_start(out=x_t, in_=hid_d[i * 128:(i + 1) * 128, :])
                xhat = workp.tile([128, c.D], F32R, name="xhat", tag="row2")
                layer_norm_tile(x_t, xhat, i)
                for d in range(c.ND):
                    transpose_128(
                        xhat[:, d * 128:(d + 1) * 128],
                        xlnT[:, d, i * 128:(i + 1) * 128],
                        xlnT8[:, d, i * 128:(i + 1) * 128],
                    )

            # V (natural layout) + kv masking + ones column
            for c0, w in tiles_of(c.D):
                wts = []
                for d in range(c.ND):
                    wv = wcache.tile([128, 512], BF16, name="wv", tag="wc")
                    nc.sync.dma_start(
                        out=wv[:, :w],
                        in_=wv_d[d * 128:(d + 1) * 128, c0:c0 + w],
                    )
                    wts.append(wv)
                h0 = c0 // 64
                nh = w // 64
                for i in range(c.NKB):
                    ps = pb.tile([128, 512], F32, name="ps_v", tag="pb")
                    for d in range(c.ND):
                        nc.tensor.matmul(
                            ps[:, :w],
                            lhsT=(xlnT[:, d, i * 128:(i + 1) * 128]),
                            rhs=(wts[d][:, :w]),
                            start=(d == 0),
                            stop=(d == c.ND - 1),
                        )
                    nc.scalar.activation(
                        out=V[:, i, h0:h0 + nh, 0:64],
                        in_=ps[:, :w].rearrange("p (h x) -> p h x", x=64),
                        func=AF.Copy,
                        scale=kvm_sb[:, i:i + 1],
                    )
            for i in range(c.NKB):
                nc.vector.tensor_scalar_mul(
                    out=V[:, i, :, 64:65],
                    in0=ones_h,
                    scalar1=kvm_sb[:, i:i + 1],
                )

            # -------- pools: phase B (attention) set up early so per-group
            # attention interleaves with Q/K production (Act exp overlaps PE)
            ytp = tc.alloc_tile_pool(name=f"ytp{rep}", bufs=1, side="right")
            expp = tc.alloc_tile_pool(name=f"expp{rep}", bufs=3, side="right")
            ynp = tc.alloc_tile_pool(name=f"ynp{rep}", bufs=2, side="right")
            drp = tc.alloc_tile_pool(name=f"drp{rep}", bufs=1, space="DRAM")
            dscr = drp.tile([c.H, c.Sq], F32R, name=f"dscr{rep}")
            yT = ytp.tile([128, c.NCC, c.Sq], BF16, name="yT")
            for cc, p0, off, cnt in seg_rows(65 * c.H, c.NCC * 128 - 65 * c.H):
                nc.sync.dma_start(
                    out=yT[p0:p0 + cnt, cc, :],
                    in_=wproj_d[65 * c.H + off:65 * c.H + off + cnt, 0:c.Sq],
                )

            # exp scale folds 1/sqrt(DH) and the fp8 dequant 1/F_QK^2
            exp_scale = 1.0 / (math.sqrt(DH) * (F_QK * F_QK if QK_FP8 else 1.0))
            fill0 = nc.gpsimd.to_reg(0.0)

            def scores_head(h):
                pbase = 64 * (h % 2)
                o = h // 2
                eT = expp.tile([128, c.NKB, c.Sq], BF16, name="eT", tag="eT")
                for kb in range(c.NKB):
                    off = 128 * kb if kb < c.NDIAG else 0
                    sps = pb.tile([128, 512], F32, name="sps", tag="pb")
                    nc.tensor.matmul(
                        sps[:, off:c.Sq],
                        lhsT=KT[pbase:pbase + 64, o,
                                kb * 128:(kb + 1) * 128],
                        rhs=QT[pbase:pbase + 64, o, off:c.Sq],
                        start=True,
                        stop=True,
                    )
                    if off:
                        # cols [0:off) are entirely below the causal diagonal
                        # and are not computed; AV reads the full width
                        nc.vector.memset(eT[:, kb, 0:off], 0.0)
                    nc.scalar.activation(
                        out=eT[:, kb, off:c.Sq],
                        in_=sps[:, off:c.Sq],
                        func=AF.Exp,
                        scale=exp_scale,
                    )
                    if kb < c.NDIAG:
                        # zero the lower-triangular part of the 128-wide
                        # diagonal sub-block (cols off..off+128)
                        nc.gpsimd.affine_select(
                            out=eT[:, kb, off:off + 128],
                            in_=eT[:, kb, off:off + 128],
                            pattern=[[1, 128]],
                            compare_op=ALU.is_ge,
                            fill=fill0,
                            base=0,
                            channel_multiplier=-1,
                        )
                return eT

            def av_head(h, eT):
                yps = pb.tile([128, 512], F32, name="yps", tag="pb")
                av_order = list(range(c.NDIAG, c.NKB)) + list(range(c.NDIAG))
                for n_i, kb in enumerate(av_order):
                    nc.tensor.matmul(
                        yps[0:65, :c.Sq],
                        lhsT=V[:, kb, h, :],
                        rhs=eT[:, kb, :],
                        start=(n_i == 0),
                        stop=(n_i == c.NKB - 1),
                    )
                # normalize: rows 0..63 / row 64
                yTu = ynp.tile([65, c.Sq], F32R, name="yTu", tag="yTu")
                nc.vector.tensor_copy(out=yTu, in_=yps[0:65, :c.Sq])
                nc.vector.reciprocal(out=yTu[64:65, :], in_=yps[64:65, :c.Sq])
                rbs = ynp.tile([65, c.Sq], F32R, name="rbs", tag="rbs")
                nc.sync.dma_start(out=dscr[h:h + 1, :], in_=yTu[64:65, :])
                _src = dscr[h:h + 1, :]
                nc.gpsimd.dma_start(
                    out=rbs,
                    in_=bass.AP(
                        tensor=_src.tensor,
                        offset=_src.offset,
                        ap=[[0, 65]] + list(_src.ap[1:]),
                    ),
                )
                yTn = ynp.tile([65, c.Sq], F32R, name="yTn", tag="yTn")
                nc.vector.tensor_mul(
                    out=yTn[0:65, :], in0=yTu[0:65, :], in1=rbs[0:65, :]
                )
                yTnb = ynp.tile([65, c.Sq], BF16, name="yTnb", tag="yTnb")
                nc.vector.tensor_copy(out=yTnb, in_=yTn)
                if has_bv:
                    nc.vector.tensor_scalar_add(
                        out=yTnb[0:64, :],
                        in0=yTnb[0:64, :],
                        scalar1=bv_sb[:, h:h + 1],
                    )
                for cc, p0, off, cnt in seg_rows(65 * h, 65):
                    nc.gpsimd.dma_start(
                        out=yT[p0:p0 + cnt, cc, :],
                        in_=yTnb[off:off + cnt, :],
                    )

            # Q and K (transposed outputs) via fp8 DoubleRow (2 contraction
            # chunks per pass at 2x rate), interleaved with attention per
            # 128-feature output group (= 2 heads).
            NJ = c.ND // 2
            for g0, gw in tiles_of(c.D):
                wqk_tiles = {}
                for which, base8 in (("q", 0), ("k", c.D)):
                    wts = []
                    if QK_FP8:
                        for j in range(NJ):
                            w8 = wcache.tile([128, 2, 512], F8,
                                             name=f"w8_{which}", tag="wc")
                            nc.sync.dma_start(
                                out=w8[:, :, :gw],
                                in_=wqk8_d[j * 128:(j + 1) * 128, :,
                                           base8 + g0: base8 + g0 + gw],
                            )
                            wts.append(w8)
                    else:
                        for d in range(c.ND):
                            w2 = wcache.tile([128, 512], BF16,
                                             name=f"w2_{which}", tag="wc")
                            nc.sync.dma_start(
                                out=w2[:, :gw],
                                in_=wqk8_d[d * 128:(d + 1) * 128,
                                           base8 + g0: base8 + g0 + gw],
                            )
                            wts.append(w2)
                    wqk_tiles[which] = wts
                for jo in range(gw // 128):
                    o = g0 // 128 + jo
                    for which, dst, ncol in (("q", QT, c.Sq), ("k", KT, c.Skv)):
                        wts = wqk_tiles[which]
                        for c0, w in tiles_of(ncol):
                            # use the transpose pool's PSUM banks (idle here)
                            # so QK production isn't gated by the exp-draining
                            # scores/yps rotation in pb
                            ps = pt.tile([128, 512], F32, name="ps_qk",
                                         tag="ps_t")
                            if QK_FP8:
                                for j in range(NJ):
                                    nc.tensor.matmul(
                                        ps[:, :w],
                                        lhsT=(wts[j][:, :,
                                                     jo * 128:(jo + 1) * 128]),
                                        rhs=(xlnT8[:, 2 * j:2 * j + 2,
                                                   c0:c0 + w]),
                                        start=(j == 0),
                                        stop=(j == NJ - 1),
                                        perf_mode=mybir.MatmulPerfMode.DoubleRow,
                                    )
                            else:
                                for d in range(c.ND):
                                    nc.tensor.matmul(
                                        ps[:, :w],
                                        lhsT=(wts[d][:, jo * 128:(jo + 1) * 128]),
                                        rhs=(xlnT[:, d, c0:c0 + w]),
                                        start=(d == 0),
                                        stop=(d == c.ND - 1),
                                    )
                            nc.vector.tensor_copy(
                                out=dst[:, o, c0:c0 + w],
                                in_=ps[:, :w],
                            )
                            if has_bqk:
                                col = o if which == "q" else c.ND + o
                                nc.vector.tensor_scalar_add(
                                    out=dst[:, o, c0:c0 + w],
                                    in0=dst[:, o, c0:c0 + w],
                                    scalar1=bqk_sb[:, col:col + 1],
                                )
                    e0 = scores_head(2 * o)
                    e1 = scores_head(2 * o + 1)
                    av_head(2 * o, e0)
                    av_head(2 * o + 1, e1)

            if DBG:
                dbgt = tc.alloc_tile_pool(name=f"dbgt{rep}", bufs=1)
                for key, tile_ap in (("qt", QT[:, 0, 0:512]), ("kt", KT[:, 0, 0:512]),
                                     ("xl", xlnT[:, 0, 0:512]), ("x8", xlnT8[:, 0, 0:512]),
                                     ("v0", V[:, 0, :, :].rearrange("p h x -> p (h x)")[:, 0:65])):
                    t32 = dbgt.tile([128, 512], F32, name=f"dbg32_{key}", tag="d32")
                    w_ = tile_ap.ap[-1][-1] if key != "v0" else 65
                    nc.vector.tensor_copy(out=t32[:, :w_], in_=tile_ap)
                    nc.sync.dma_start(out=dbg[key], in_=t32[:, :dbg[key].ap[-1][-1]])
                dbgt.release()

            xlp.release()  # xlnT dead

            drp.release()
            ynp.release()
            expp.release()
            qkvp.release()  # QT/KT/V dead

            # -------- pools: phase C..D --------
            hzp = tc.alloc_tile_pool(name=f"hzp{rep}", bufs=1)
            h_sb = hzp.tile([128, c.NQB, c.D], F32, name="h_sb")
            zT = hzp.tile([128, c.ND, c.Sq], BF16, name="zT")

            # ---- Phase C: proj + residual + LN2 + transpose ----
            for c0, w in tiles_of(c.D):
                wts = []
                for cc in range(c.NCC):
                    wp = wcache.tile([128, 512], BF16, name="wp", tag="wc")
                    nc.sync.dma_start(
                        out=wp[:, :w],
                        in_=wproj_d[cc * 128:(cc + 1) * 128, c0:c0 + w],
                    )
                    wts.append(wp)
                for qb in range(c.NQB):
                    ps = pb.tile([128, 512], F32, name="ps_p", tag="pb")
                    for cc in range(c.NCC):
                        nc.tensor.matmul(
                            ps[:, :w],
                            lhsT=(yT[:, cc, qb * 128:(qb + 1) * 128]),
                            rhs=(wts[cc][:, :w]),
                            start=(cc == 0),
                            stop=(cc == c.NCC - 1),
                        )
                    nc.vector.tensor_add(
                        out=h_sb[:, qb, c0:c0 + w],
                        in0=ps[:, :w],
                        in1=resid[:, qb, c0:c0 + w],
                    )

            if DBG:
                nc.sync.dma_start(out=dbg["h"], in_=h_sb[:, 0, :])
                nc.sync.dma_start(out=dbg["resid"], in_=resid[:, 0, :])
                dbgt3 = tc.alloc_tile_pool(name=f"dbgt3{rep}", bufs=1)
                ty = dbgt3.tile([128, 11, 512], F32, name="dbg_yt32")
                nc.vector.tensor_copy(out=ty, in_=yT)
                nc.sync.dma_start(out=dbg["yt"], in_=ty)
                dbgt3.release()

            ytp.release()  # yT dead
            residp.release()

            for qb in range(c.NQB):
                xhat2 = workp.tile([128, c.D], F32R, name="xhat2", tag="row2")
                layer_norm_tile(h_sb[:, qb, :], xhat2, 100 + qb)
                for d in range(c.ND):
                    transpose_128(
                        xhat2[:, d * 128:(d + 1) * 128],
                        zT[:, d, qb * 128:(qb + 1) * 128],
                    )

            # ---- Phase D: MLP ----
            gp = tc.alloc_tile_pool(name=f"gp{rep}", bufs=1)
            outp = tc.alloc_tile_pool(name=f"outp{rep}", bufs=1)
            w2p = tc.alloc_tile_pool(name=f"w2p{rep}", bufs=6)
            gT = gp.tile([128, c.NHT, c.Sq], BF16, name="gT")

            for g0, gw in tiles_of(c.Dff):
                wts = []
                for d in range(c.ND):
                    wf = wcache.tile([128, 512], BF16, name="wf", tag="wc")
                    nc.sync.dma_start(
                        out=wf[:, :gw],
                        in_=wfc_d[d * 128:(d + 1) * 128, g0:g0 + gw],
                    )
                    wts.append(wf)
                for j in range(gw // 128):
                    ht = g0 // 128 + j
                    ps = pb.tile([128, 512], F32, name="ps_f", tag="pb")
                    for d in range(c.ND):
                        nc.tensor.matmul(
                            ps[:, :c.Sq],
                            lhsT=(wts[d][:, j * 128:(j + 1) * 128]),
                            rhs=(zT[:, d, :]),
                            start=(d == 0),
                            stop=(d == c.ND - 1),
                        )
                    nc.scalar.activation(
                        out=gT[:, ht, :],
                        in_=ps[:, :c.Sq],
                        func=AF.Gelu_apprx_tanh,
                        bias=bfc_sb[:, ht:ht + 1],
                    )

            out_ts = []
            for qs in range(c.NQB):
                ot = outp.tile([128, c.D], F32, name=f"out_{qs}")
                out_ts.append(ot)
            for c0, w in tiles_of(c.D):
                psms = [
                    pb.tile([128, 512], F32, name=f"ps_m_{qs}", tag="pb")
                    for qs in range(c.NQB)
                ]
                for ht in range(c.NHT):
                    wf2 = w2p.tile([128, 512], BF16, name="wf2", tag="wf2")
                    nc.sync.dma_start(
                        out=wf2[:, :w],
                        in_=wfc2_d[ht * 128:(ht + 1) * 128, c0:c0 + w],
                    )
                    for qs in range(c.NQB):
                        nc.tensor.matmul(
                            psms[qs][:, :w],
                            lhsT=gT[:, ht, qs * 128:(qs + 1) * 128],
                            rhs=wf2[:, :w],
                            start=(ht == 0),
                            stop=(ht == c.NHT - 1),
                        )
                for qs in range(c.NQB):
                    nc.vector.tensor_add(
                        out=out_ts[qs][:, c0:c0 + w],
                        in0=psms[qs][:, :w],
                        in1=h_sb[:, qs, c0:c0 + w],
                    )
            for qs in range(c.NQB):
                if has_bfc2:
                    nc.vector.tensor_add(
                        out=out_ts[qs], in0=out_ts[qs], in1=bfc2_sb
                    )
                nc.sync.dma_start(
                    out=out_d[qs * 128:(qs + 1) * 128, :],
                    in_=out_ts[qs],
                )

            w2p.release()
            outp.release()
            gp.release()
            hzp.release()

        for _rep in range(repeat):
            _emit_phases(_rep)

        pt.release()
        pb.release()
        wcache.release()
        statp.release()
        workp.release()
        constp.release()

    nc.compile()
    return nc


# ----------------------------------------------------------------------------
# Host-side preparation
# ----------------------------------------------------------------------------

def prep_inputs(cfg: Cfg, hidden_states, attention_mask, ln1_g, ln1_b, w_qkv,
                b_qkv, w_proj, b_proj, ln2_g, ln2_b, w_fc, b_fc, w_fc2, b_fc2):
    """Build per-core in_maps. Returns (in_maps, flags)."""
    c = cfg
    B = hidden_states.shape[0]
    f32 = np.float32
    import ml_dtypes
    bf16 = ml_dtypes.bfloat16
    f8 = ml_dtypes.float8_e4m3

    # fold LN affine params into following matmuls
    wqkv_f = (ln1_g[:, None] * w_qkv).astype(f32)
    bqkv_f = (ln1_b @ w_qkv + b_qkv).astype(f32)
    wfc_f = (ln2_g[:, None] * w_fc).astype(f32)
    bfc_f = (ln2_b @ w_fc + b_fc).astype(f32)

    if QK_FP8:
        # q/k weights: fp8 DoubleRow layout [ND/2*128, 2, 2D]
        # wqk8[128j+p, i, col] = S_W8 * wqkv[256j + 128i + p, col], col < 2D
        wqk = (wqkv_f[:, :2 * c.D] * S_W8).reshape(c.ND // 2, 2, 128, 2 * c.D)
        wqk8 = np.ascontiguousarray(wqk.transpose(0, 2, 1, 3)).reshape(
            c.ND // 2 * 128, 2, 2 * c.D).astype(f8)

    # proj weight strip with per-head ones rows carrying b_proj/H
    wproj_p = np.zeros((c.NCC * 128, c.D), f32)
    for h in range(c.H):
        wproj_p[65 * h:65 * h + 64, :] = w_proj[64 * h:64 * h + 64, :]
        wproj_p[65 * h + 64, :] = b_proj / c.H

    bq = bqkv_f[0:c.D]
    bk = bqkv_f[c.D:2 * c.D]
    bv = bqkv_f[2 * c.D:3 * c.D]

    has_bqk = bool(np.any(bq) or np.any(bk))
    has_bv = bool(np.any(bv))
    has_bfc2 = bool(np.any(b_fc2))

    shared = {
        "wv": wqkv_f[:, 2 * c.D:].astype(bf16),
        "wproj": wproj_p.astype(bf16),
        "wfc": wfc_f.astype(bf16),
        "wfc2": np.asarray(w_fc2).astype(bf16),
        "bfc": bfc_f.reshape(c.NHT, 128),
        "ident": np.eye(128, dtype=f32),
    }
    if QK_FP8:
        shared["wqk8"] = wqk8
    else:
        shared["wqk"] = wqkv_f[:, :2 * c.D].astype(bf16)
    if has_bqk:
        # QT/KT carry a factor of F_QK; biases must too
        shared["bqk"] = np.concatenate(
            [bq.reshape(c.ND, 128), bk.reshape(c.ND, 128)], axis=0
        ) * (F_QK if QK_FP8 else 1.0)
    if has_bv:
        shared["bv"] = bv.reshape(c.H, 64)
    if has_bfc2:
        shared["bfc2"] = b_fc2.reshape(1, c.D).astype(f32)

    amask = np.asarray(attention_mask).astype(f32)
    in_maps = []
    for core in range(2 * B):
        b, g = divmod(core, 2)
        q0 = g * c.Sq
        hid = np.zeros((c.Skv, c.D), f32)
        kvm = np.zeros((c.Skv,), f32)
        hid[0:c.Sq] = hidden_states[b, q0:q0 + c.Sq]
        kvm[0:c.Sq] = amask[b, q0:q0 + c.Sq]
        if q0 > 0:
            hid[c.Sq:c.Sq + q0] = hidden_states[b, 0:q0]
            kvm[c.Sq:c.Sq + q0] = amask[b, 0:q0]
        m = dict(shared)
        m["hid"] = hid
        m["kvmask"] = kvm.reshape(c.NKB, 128)
        in_maps.append(m)
    return in_maps, (has_bqk, has_bv, has_bfc2)


_CACHE = {}
LAST_RESULTS = None


def kernel(**inputs):
    global LAST_RESULTS
    import concourse.bass_utils as bass_utils

    cfg = Cfg()
    inputs = {k: np.asarray(v) for k, v in inputs.items()}
    B = inputs["hidden_states"].shape[0]
    in_maps, flags = prep_inputs(cfg, **inputs)

    key = (cfg, flags)
    if key not in _CACHE:
        _CACHE[key] = build_program(cfg, *flags)
    nc = _CACHE[key]

    res = bass_utils.run_bass_kernel_spmd(
        nc, in_maps, core_ids=list(range(2 * B))
    )
    LAST_RESULTS = res

    out = np.zeros((B, 2 * cfg.Sq, cfg.D), np.float32)
    for core in range(2 * B):
        b, g = divmod(core, 2)
        out[b, g * cfg.Sq:(g + 1) * cfg.Sq] = res.results[core]["out"]
    return out
